# revision 1
# baseline (speedup 1.0000x reference)
"""Trainium2 Bass kernel for 3-layer GraphSAGE encoder (nn_Encoder_38757784879702).

Strategy (8 NeuronCores, node-partitioned / graph parallel):
  - Nodes sharded 12500/core (padded to 12544 = 98 tiles of 128).
  - Edges assigned to the core owning dst, sorted by dst, padded into
    uniform 128-edge chunks (K_C chunks per dst-tile) so the SPMD program
    is identical across cores.
  - Mean aggregation = one-hot matmul on TensorE: for each chunk,
    psum[dst,:] += onehot[e,dst].T @ gathered[e,:], with 1/deg folded into
    the host-built one-hot blocks.
  - Transform-first: z_l = y_{l-1} @ Wl_l.T is computed per-shard, then
    AllGather'd so every core can gather any z row for its edges.
  - Neighbor rows fetched with gpsimd indirect DMA (int32 row ids,
    128*K_C rows per instruction).
  - PReLU(m) = max(m, a*m) on VectorE (valid since a = 0.25 < 1; checked
    on host).
"""

import sys

sys.path.insert(0, "/opt/trn_rl_repo")

import numpy as np

import concourse.bass as bass
import concourse.bacc as bacc
import concourse.mybir as mybir
import concourse.tile as tile
from concourse.bass_utils import run_bass_kernel_spmd
from concourse.masks import make_identity

F32 = mybir.dt.float32
BF16 = mybir.dt.bfloat16
F32R = mybir.dt.float32r
I32 = mybir.dt.int32

# ---------------------------------------------------------------------------
# Problem geometry (hardcoded: harness contract)
N_NODES = 100000
N_EDGES = 800000
D_IN = 128
D_H = 512
N_CORES = 8

# table dtype ("f32"|"bf16") for gather tables / one-hot; matmul cast mode
# ("f32"|"f32r"|"bf16") for the dense y@W matmuls; feature splits per layer
# table (1|2|4) to overlap AllGather halves with aggregation sweeps.
CONFIG = {"TD": "bf16", "MM": "f32r", "NSPLIT": 1, "CAG": False}


def _dt(s):
    return {"f32": F32, "bf16": BF16}[s]


def _npdt(s):
    return {"f32": np.float32, "bf16": None}[s]


class Plan:
    """All host-derived geometry + per-core arrays."""

    def __init__(self, n_nodes, n_cores, d_in, d_h, cfg):
        self.cfg = cfg
        self.N = n_nodes
        self.C = n_cores
        self.D_IN = d_in
        self.D_H = d_h
        assert n_nodes % n_cores == 0
        self.NSH = n_nodes // n_cores              # real nodes per core
        self.NT = -(-self.NSH // 128)              # dst tiles per core
        self.SH = self.NT * 128                    # padded nodes per core
        self.NR = self.C * self.SH                 # padded global rows
        self.K_C = None                            # chunks per tile (from data)
        # node blocks (chunked AllGather): split tiles into up to 4 groups
        nbk = min(4, self.NT)
        q, r = divmod(self.NT, nbk)
        self.BT = [q + (1 if i < r else 0) for i in range(nbk)]   # tiles per block
        self.TS = [sum(self.BT[:i]) for i in range(nbk)]          # tile start
        self.BS = [bt * 128 for bt in self.BT]                    # rows/core/block
        self.GB = [self.C * sum(self.BS[:i]) for i in range(nbk)] # global row base
        self.NBK = nbk


def _zrow(plan, n):
    r = n // plan.NSH
    loc = n % plan.NSH
    t = loc // 128
    b = np.searchsorted(np.cumsum(plan.BT), t, side="right")
    b = np.minimum(b, plan.NBK - 1)
    gb = np.asarray(plan.GB)[b]
    bs = np.asarray(plan.BS)[b]
    ts = np.asarray(plan.TS)[b]
    return gb + r * bs + (loc - ts * 128)


def preprocess(plan, x, edge_index, weights):
    """Build per-core input maps (numpy only)."""
    cfg = plan.cfg
    td = cfg["TD"]
    tdnp = np.float32 if td == "f32" else None  # bf16 handled via ml_dtypes
    if td == "bf16":
        import ml_dtypes
        tdnp = ml_dtypes.bfloat16
    mmnp = {"f32": np.float32, "f32r": np.float32}.get(cfg["MM"])
    if mmnp is None:
        import ml_dtypes
        mmnp = ml_dtypes.bfloat16

    N, C, NSH, SH, NT = plan.N, plan.C, plan.NSH, plan.SH, plan.NT
    src = np.asarray(edge_index[0], dtype=np.int64)
    dst = np.asarray(edge_index[1], dtype=np.int64)
    x = np.asarray(x, dtype=np.float32)

    deg = np.bincount(dst, minlength=N)
    invdeg = (1.0 / np.maximum(deg, 1)).astype(np.float32)

    # sort globally by (owning core, dst tile, serpentine zrow) so each
    # (core, tile) group is contiguous; odd tiles are sorted descending by
    # zrow so the row profile is continuous across tile boundaries and
    # gather windows can span tiles
    zr_all = _zrow(plan, src)
    core_all = dst // NSH
    tile_all = (dst % NSH) // 128
    serp = np.where(tile_all % 2 == 0, zr_all, plan.NR - 1 - zr_all)
    order = np.lexsort((serp, tile_all, core_all))
    s_src, s_dst = src[order], dst[order]
    s_zr = zr_all[order]
    core_of = core_all[order]
    tile_of = tile_all[order]

    gkey = core_of * NT + tile_of
    cnt = np.bincount(gkey, minlength=C * NT)
    plan.K_C = int(-(-cnt.max() // 128))
    K_C = plan.K_C
    WR = plan.WR = min(32768, plan.NR)            # window rows (int16 limit)
    NRR = plan.NR

    starts = np.cumsum(cnt) - cnt
    rank = np.arange(len(s_dst)) - starts[gkey]
    p = rank % 128
    k = rank // 128
    col = tile_of * K_C + k                       # chunk column index
    dloc = (s_dst % NSH) - tile_of * 128          # 0..127 within tile

    # cross-core [lo, hi] zrow union per (tile, chunk)
    lo = np.full((NT, K_C), NRR * 2, np.int64)
    hi = np.full((NT, K_C), -1, np.int64)
    np.minimum.at(lo, (tile_of, k), s_zr)
    np.maximum.at(hi, (tile_of, k), s_zr)

    # greedy static windows over the global serpentine chunk sequence:
    # merge consecutive chunks (across tile boundaries) while the union
    # fits one WR-row table window, capped at NCHMAX chunks per call
    NCHMAX = plan.NCHMAX = 8
    NC_ALL = NT * K_C
    flo = lo.reshape(NC_ALL)
    fhi = hi.reshape(NC_ALL)
    plan.windows = []                             # global list of (c0, nch, base)
    plan.chunk2win = {}
    kk = 0
    while kk < NC_ALL:
        clo, chi = flo[kk], fhi[kk]
        n = 1
        while kk + n < NC_ALL and n < NCHMAX:
            nlo = min(clo, flo[kk + n])
            nhi = max(chi, fhi[kk + n])
            b = min(nlo, NRR - WR) if nhi >= 0 else 0
            if nhi - b <= WR - 1 or nhi < 0:
                clo, chi, n = nlo, nhi, n + 1
            else:
                break
        if chi < 0:
            b = 0
        else:
            b = max(0, min(clo, NRR - WR))
            assert chi - b <= WR - 1, "single chunk exceeds int16 window"
        wi = len(plan.windows)
        plan.windows.append((int(kk), int(n), int(b)))
        for c in range(kk, kk + n):
            plan.chunk2win[c] = (wi, c - kk)
        kk += n

    # base per chunk for idx relativization
    cbase_flat = np.zeros(NC_ALL, np.int64)
    for (c0, n, b) in plan.windows:
        cbase_flat[c0:c0 + n] = b
    cbase = cbase_flat.reshape(NT, K_C)
    rel = s_zr - cbase[tile_of, k]
    assert rel.min() >= 0 and rel.max() < WR

    # idx stream layout: chunk (t, k) owns 8 int16 columns at (t*K_C+k)*8;
    # stream element j -> partition j%16 (replicated across the 8 groups of
    # 16 partitions), column j//16.
    idx_all = np.zeros((C, 16, NT * K_C * 8), np.int16)
    j_in_c = rank % 128
    icol = (tile_of * K_C + k) * 8 + j_in_c // 16
    idx_all[core_of, j_in_c % 16, icol] = rel.astype(np.int16)
    idx_all = np.tile(idx_all, (1, 8, 1))         # replicate to 128 partitions

    oh_all = np.zeros((C, 128, NT * K_C * 128), np.float32)
    oh_all[core_of, p, col * 128 + dloc] = invdeg[s_dst]
    oh_all = oh_all.astype(tdnp)

    # x shard (padded, f32) for root/residual terms; TD copy for gather AG
    xsh = np.zeros((C, SH, plan.D_IN), np.float32)
    xs = x.reshape(C, NSH, plan.D_IN)
    xsh[:, :NSH, :] = xs

    def wt_blocks(w, npdt):
        # W [O, I] -> blocks [128, (I/128)*O], block k = W.T[k*128:(k+1)*128, :]
        wt = np.ascontiguousarray(w.T.astype(np.float32))  # [I, O]
        i, o = wt.shape
        return np.ascontiguousarray(
            wt.reshape(i // 128, 128, o).transpose(1, 0, 2).reshape(128, (i // 128) * o)
        ).astype(npdt)

    a_val = float(np.asarray(weights["a"]))
    assert 0.0 <= a_val <= 1.0, "prelu max-trick requires 0<=a<=1"

    common = {
        "oh": None,  # per-core below
        "idx": None,
        "wl1t": wt_blocks(weights["Wl1"], mmnp),
        "wr1t": wt_blocks(weights["Wr1"], mmnp),
        "wwt": wt_blocks(weights["Ww"], mmnp),
        "ww2t": wt_blocks(weights["Ww2"], mmnp),
        "wl2t": wt_blocks(weights["Wl2"], mmnp),
        "wr2t": wt_blocks(weights["Wr2"], mmnp),
        "wl3t": wt_blocks(weights["Wl3"], mmnp),
        "wr3t": wt_blocks(weights["Wr3"], mmnp),
        "bl1": np.asarray(weights["bl1"], np.float32).reshape(1, -1).astype(mmnp),
        "bw": np.asarray(weights["bw"], np.float32).reshape(1, -1).astype(mmnp),
        "bw2": np.asarray(weights["bw2"], np.float32).reshape(1, -1).astype(mmnp),
        "bl2": np.asarray(weights["bl2"], np.float32).reshape(1, -1).astype(mmnp),
        "bl3": np.asarray(weights["bl3"], np.float32).reshape(1, -1).astype(mmnp),
        "a_bc": np.full((128, 1), a_val, np.float32),
        "ones_in": np.ones((1, 128), np.float32).astype(mmnp),
    }
    in_maps = []
    for c in range(C):
        m = dict(common)
        m["oh"] = np.ascontiguousarray(oh_all[c])
        m["idx"] = np.ascontiguousarray(idx_all[c])
        m["x_sh"] = np.ascontiguousarray(xsh[c])
        m["xg_sh"] = np.ascontiguousarray(xsh[c].astype(tdnp))
        in_maps.append(m)
    return in_maps


def build_program(plan):
    """Emit the SPMD Bass/Tile program (identical for every core)."""
    cfg = plan.cfg
    MM = cfg["MM"]
    # operand declaration dtypes: fp32r matmul operands must be *declared*
    # fp32r end-to-end (BIR verifier rejects plain-f32 producers feeding an
    # fp32r matmul); byte layout is identical to f32 so host arrays stay f32.
    MMD = {"f32": F32, "f32r": F32R, "bf16": BF16}[MM]
    if cfg["TD"] == "bf16":
        TD = BF16
    else:
        TD = F32 if MM == "f32" else F32R
    NS = cfg["NSPLIT"]
    NT, SH, NR, K_C = plan.NT, plan.SH, plan.NR, plan.K_C
    WR, windows = plan.WR, plan.windows
    chunk2win, NCHMAX = plan.chunk2win, plan.NCHMAX
    I16 = mybir.dt.int16
    D_I, D_Hh = plan.D_IN, plan.D_H
    DS = D_Hh // NS
    KB = D_Hh // 128  # K blocks for dense 512-dim matmuls
    KBI = D_I // 128
    RG = [list(range(plan.C))]

    NBK, BT, TS, BS, GB = plan.NBK, plan.BT, plan.TS, plan.BS, plan.GB
    # chunked AllGathers write disjoint slices of one Shared tensor; the
    # interpreter's pair-aliasing model only allows a single writer per
    # Shared tensor, so sim runs use addr_space=Local instead.
    ADDR = "Shared" if cfg.get("CAG", True) else "Local"
    nc = bacc.Bacc("TRN2", target_bir_lowering=False, debug=False,
                   enable_asserts=False, num_devices=plan.C,
                   num_swdge_queues=4)
    qctr = [0]
    def next_q():
        qctr[0] += 1
        return qctr[0] % 4

    # --- I/O ----------------------------------------------------------------
    x_sh = nc.declare_dram_parameter("x_sh", [SH, D_I], F32, isOutput=False)
    xg_sh = nc.declare_dram_parameter("xg_sh", [SH, D_I], TD, isOutput=False)
    oh = nc.declare_dram_parameter("oh", [128, NT * K_C * 128], TD, isOutput=False)
    idx = nc.declare_dram_parameter("idx", [128, NT * K_C * 8], I16, isOutput=False)
    wnames = ["wl1t", "wr1t", "wwt", "ww2t"]
    wins = {n: nc.declare_dram_parameter(n, [128, KBI * D_Hh], MMD, isOutput=False)
            for n in wnames}
    for n in ["wl2t", "wr2t", "wl3t", "wr3t"]:
        wins[n] = nc.declare_dram_parameter(n, [128, KB * D_Hh], MMD, isOutput=False)
    bnames = ["bl1", "bw", "bw2", "bl2", "bl3"]
    bins = {n: nc.declare_dram_parameter(n, [1, D_Hh], MMD, isOutput=False)
            for n in bnames}
    a_bc = nc.declare_dram_parameter("a_bc", [128, 1], F32, isOutput=False)
    ones_in = nc.declare_dram_parameter("ones_in", [1, 128], MMD, isOutput=False)
    h3_out = nc.declare_dram_parameter("h3", [SH, D_Hh], F32, isOutput=True)

    with tile.TileContext(nc) as tc:
        with (
            tc.tile_pool(name="dram", bufs=1, space="DRAM") as dpool,
            tc.tile_pool(name="const", bufs=1) as cpool,
            tc.tile_pool(name="gin", bufs=5) as gpool,
            tc.tile_pool(name="ohp", bufs=3) as ohpool,
            tc.tile_pool(name="work", bufs=2) as wk,
            tc.tile_pool(name="psA", bufs=2, space="PSUM") as psA,
            tc.tile_pool(name="psB", bufs=3, space="PSUM") as psB,
            tc.tile_pool(name="psT", bufs=2, space="PSUM") as psT,
        ):
            # --- internal DRAM ---------------------------------------------
            xg_loc = {b: dpool.tile([BS[b], D_I], TD, name=f"xg_loc{b}")
                      for b in range(NBK)}
            xg_full = dpool.tile([NR, D_I], TD, name="xg_full", addr_space=ADDR)
            z_loc = {}
            z_full = {}
            for l in (2, 3):
                z_full[l] = dpool.tile([NR, D_Hh], TD, name=f"z{l}full",
                                       addr_space=ADDR)
                for b in range(NBK):
                    z_loc[(l, b)] = dpool.tile([BS[b], D_Hh], TD,
                                               name=f"z{l}loc{b}")
            w2_d = dpool.tile([SH, D_Hh], F32, name="w2_d")
            rt2_d = dpool.tile([SH, D_Hh], F32, name="rt2_d")
            rt3_d = dpool.tile([SH, D_Hh], F32, name="rt3_d")
            y2_d = dpool.tile([SH, D_Hh], F32, name="y2_d") if NS > 1 else None

            # --- persistent SBUF -------------------------------------------
            ident = cpool.tile([128, 128], F32, name="ident")
            make_identity(nc, ident[:])
            ones1 = cpool.tile([1, 128], MMD, name="ones1")
            nc.sync.dma_start(out=ones1[:], in_=ones_in[:])
            a_sb = cpool.tile([128, 1], F32, name="a_sb")
            nc.sync.dma_start(out=a_sb[:], in_=a_bc[:])
            idx_sb = cpool.tile([128, NT * K_C * 8], I16, name="idx_sb")
            nc.sync.dma_start(out=idx_sb[:], in_=idx[:])
            wsb = {}
            for n, hh in wins.items():
                kb = KBI if n in wnames else KB
                wsb[n] = cpool.tile([128, kb * D_Hh], MMD, name=f"{n}_sb")
                nc.sync.dma_start(out=wsb[n][:], in_=hh[:])
            bsb = {}
            for n, hh in bins.items():
                bsb[n] = cpool.tile([1, D_Hh], MMD, name=f"{n}_sb")
                nc.sync.dma_start(out=bsb[n][:], in_=hh[:])

            # --- gather-table AllGather for x (per node block) -------------
            for b in range(NBK):
                nc.sync.dma_start(
                    out=xg_loc[b][:],
                    in_=xg_sh[TS[b] * 128:TS[b] * 128 + BS[b], :])
                nc.gpsimd.collective_compute(
                    "AllGather", mybir.AluOpType.bypass, replica_groups=RG,
                    ins=[xg_loc[b][:].opt()],
                    outs=[xg_full[GB[b]:GB[b] + plan.C * BS[b], :].opt()],
                )

            # --- helpers ----------------------------------------------------
            # per-window gather tiles; a window may span tile boundaries
            g_tiles = {}

            def emit_window(wi, table, width):
                (c0, nch, b) = windows[wi]
                g = gpool.tile([128, NCHMAX, width], TD, name="g", tag="g")
                ni = nch * 128
                ic = c0 * 8
                nc.gpsimd.dma_gather(
                    out_ap=g[:, 0:nch, :],
                    in_ap=table[b:b + WR, :],
                    idxs_ap=idx_sb[:, ic:ic + ni // 16],
                    num_idxs=ni,
                    num_idxs_reg=ni,
                    elem_size=width,
                    queue_num=next_q(),
                )
                return g

            def gather(table, t, width):
                for kk in range(K_C):
                    wi, _ = chunk2win[t * K_C + kk]
                    if wi not in g_tiles:
                        g_tiles[wi] = emit_window(wi, table, width)
                return t

            def onehot_tile(t):
                o = ohpool.tile([128, K_C * 128], TD, name="oht", tag="oht")
                nc.sync.dma_start(
                    out=o[:], in_=oh[:, t * K_C * 128:(t + 1) * K_C * 128])
                return o

            def agg_psum(t, o, width):
                ps = psA.tile([128, width], F32, name="psagg", tag="psagg")
                for c in range(K_C):
                    wi, off = chunk2win[t * K_C + c]
                    nc.tensor.matmul(
                        ps[:, :], lhsT=o[:, c * 128:(c + 1) * 128],
                        rhs=g_tiles[wi][:, off, :],
                        start=(c == 0), stop=(c == K_C - 1))
                return ps

            def transpose128(in_ap, out_dt, name):
                """[128,128] SBUF f32 -> transposed SBUF tile of out_dt."""
                pt = psT.tile([128, 128], F32, name="pt", tag="pt")
                nc.tensor.transpose(out=pt[:], in_=in_ap, identity=ident[:])
                ob = wk.tile([128, 128], out_dt, name=name, tag=name)
                nc.scalar.copy(out=ob[:], in_=pt[:])
                return ob

            def dense_psum(lhsT_blocks, w_name, bias, nblk, name, keep_open=False):
                """sum_k lhsT_k.T @ W_k [+ ones.T @ bias] -> psum [128, D_H]"""
                ps = psB.tile([128, D_Hh], F32, name=name, tag="psb")
                nb = nblk + (1 if bias is not None else 0)
                i = 0
                for kk in range(nblk):
                    nc.tensor.matmul(
                        ps[:, :], lhsT=lhsT_blocks[kk][:, :],
                        rhs=wsb[w_name][:, kk * D_Hh:(kk + 1) * D_Hh],
                        start=(i == 0), stop=(i == nb - 1 and not keep_open))
                    i += 1
                if bias is not None:
                    nc.tensor.matmul(
                        ps[:, :], lhsT=ones1[:, :], rhs=bsb[bias][:, :],
                        start=(i == 0), stop=not keep_open)
                return ps

            def prelu_from(ps_or_sb, extra_sb, name):
                """h = prelu(ps + extra) (extra may be None)."""
                if extra_sb is not None:
                    t1 = wk.tile([128, ps_or_sb.shape[-1]], F32, name="t1", tag="t1")
                    nc.vector.tensor_tensor(out=t1[:], in0=ps_or_sb[:, :],
                                            in1=extra_sb[:, :],
                                            op=mybir.AluOpType.add)
                    base = t1
                else:
                    base = ps_or_sb
                t2 = wk.tile([128, base.shape[-1]], F32, name="t2", tag="t2")
                nc.vector.tensor_scalar_mul(t2[:], base[:, :], a_sb[:, 0:1])
                h = wk.tile([128, base.shape[-1]], F32, name=name, tag=name)
                nc.vector.tensor_tensor(out=h[:], in0=base[:, :], in1=t2[:],
                                        op=mybir.AluOpType.max)
                return h

            def zrt_tail(y_sb, t, wl_name, wr_name, bl_name, zl, rt_d):
                """From full-width y tile: z_{l+1} -> z_loc splits, rt -> DRAM."""
                yT = []
                for kk in range(KB):
                    yT.append(transpose128(y_sb[:, kk * 128:(kk + 1) * 128],
                                           MMD, f"yT{kk}"))
                psz = dense_psum(yT, wl_name, None, KB, "psz")
                z_sb = wk.tile([128, D_Hh], TD, name="z_sb", tag="z_sb")
                nc.scalar.copy(out=z_sb[:], in_=psz[:, :])
                bb = 0
                while bb < NBK - 1 and t >= TS[bb + 1]:
                    bb += 1
                nc.sync.dma_start(
                    out=z_loc[(zl, bb)][(t - TS[bb]) * 128:(t - TS[bb] + 1) * 128, :],
                    in_=z_sb[:, :])
                psr = dense_psum(yT, wr_name, bl_name, KB, "psr")
                rt_sb = wk.tile([128, D_Hh], F32, name="rt_sb", tag="rt_sb")
                nc.scalar.copy(out=rt_sb[:], in_=psr[:, :])
                nc.sync.dma_start(out=rt_d[t * 128:(t + 1) * 128, :], in_=rt_sb[:])

            # =================== Layer 1 ===================================
            def l1_tile(t):
                gather(xg_full, t, D_I)
                o1 = onehot_tile(t)
                psa = agg_psum(t, o1, D_I)
                # own x tile + transposes
                x_t = wk.tile([128, D_I], F32, name="x_t", tag="x_t")
                nc.sync.dma_start(out=x_t[:], in_=x_sh[t * 128:(t + 1) * 128, :])
                xT = transpose128(x_t[:, :], MMD, "xT")
                agg1 = wk.tile([128, D_I], F32, name="agg1", tag="agg1")
                nc.vector.tensor_copy(out=agg1[:], in_=psa[:, :])
                aggT = transpose128(agg1[:, :], MMD, "aggT")
                psh = dense_psum([aggT], "wl1t", None, 1, "psh", keep_open=True)
                nc.tensor.matmul(psh[:, :], lhsT=xT[:, :],
                                 rhs=wsb["wr1t"][:, :], start=False, stop=False)
                nc.tensor.matmul(psh[:, :], lhsT=ones1[:, :],
                                 rhs=bsb["bl1"][:, :], start=False, stop=True)
                h1 = prelu_from(psh, None, "h1")
                psy = dense_psum([xT], "wwt", "bw", 1, "psy")
                y1 = wk.tile([128, D_Hh], F32, name="y1", tag="y1")
                nc.vector.tensor_tensor(out=y1[:], in0=psy[:, :], in1=h1[:],
                                        op=mybir.AluOpType.add)
                psw = dense_psum([xT], "ww2t", "bw2", 1, "psw")
                w2_sb = wk.tile([128, D_Hh], F32, name="w2_sb", tag="w2_sb")
                nc.vector.tensor_tensor(out=w2_sb[:], in0=psw[:, :], in1=h1[:],
                                        op=mybir.AluOpType.add)
                nc.sync.dma_start(out=w2_d[t * 128:(t + 1) * 128, :], in_=w2_sb[:])
                zrt_tail(y1, t, "wl2t", "wr2t", "bl2", 2, rt2_d)

            def l2_tile(t):
                gather(z_full[2], t, D_Hh)
                o2 = onehot_tile(t)
                psa = agg_psum(t, o2, D_Hh)
                rt_t = wk.tile([128, D_Hh], F32, name="rt_t", tag="rt_t")
                nc.sync.dma_start(out=rt_t[:],
                                  in_=rt2_d[t * 128:(t + 1) * 128, :])
                h2 = prelu_from(psa, rt_t, "h2")
                w2_t = wk.tile([128, D_Hh], F32, name="w2_t", tag="w2_t")
                nc.sync.dma_start(out=w2_t[:],
                                  in_=w2_d[t * 128:(t + 1) * 128, :])
                y2 = wk.tile([128, D_Hh], F32, name="y2", tag="y2")
                nc.vector.tensor_tensor(out=y2[:], in0=h2[:], in1=w2_t[:],
                                        op=mybir.AluOpType.add)
                zrt_tail(y2, t, "wl3t", "wr3t", "bl3", 3, rt3_d)

            def ag_block(l, b):
                nc.gpsimd.collective_compute(
                    "AllGather", mybir.AluOpType.bypass, replica_groups=RG,
                    ins=[z_loc[(l, b)][:].opt()],
                    outs=[z_full[l][GB[b]:GB[b] + plan.C * BS[b], :].opt()])

            for b in range(NBK):
                for t in range(TS[b], TS[b] + BT[b]):
                    l1_tile(t)
                ag_block(2, b)

            g_tiles.clear()
            for b in range(NBK):
                for t in range(TS[b], TS[b] + BT[b]):
                    l2_tile(t)
                ag_block(3, b)

            g_tiles.clear()
            for t in range(NT):
                gather(z_full[3], t, D_Hh)
                o3 = onehot_tile(t)
                psa = agg_psum(t, o3, D_Hh)
                rt_t = wk.tile([128, D_Hh], F32, name="rt3_t", tag="rt_t")
                nc.sync.dma_start(out=rt_t[:],
                                  in_=rt3_d[t * 128:(t + 1) * 128, :])
                h3 = prelu_from(psa, rt_t, "h3t")
                nc.sync.dma_start(out=h3_out[t * 128:(t + 1) * 128, :],
                                  in_=h3[:])

    nc.compile()
    return nc


_CACHE = {}


def _get_program(plan):
    key = (plan.N, plan.C, plan.K_C, tuple(sorted(plan.cfg.items())))
    if key not in _CACHE:
        _CACHE[key] = build_program(plan)
    return _CACHE[key]


def run(inputs, trace=False, **rkw):
    inputs = {k: np.asarray(v) for k, v in inputs.items()}
    x = inputs["x"]
    edge_index = inputs["edge_index"]
    plan = Plan(N_NODES, N_CORES, D_IN, D_H, CONFIG)
    in_maps = preprocess(plan, x, edge_index, inputs)
    nc = _get_program(plan)
    res = run_bass_kernel_spmd(nc, in_maps, core_ids=list(range(N_CORES)),
                               trace=trace, **rkw)
    outs = [res.results[c]["h3"][:plan.NSH] for c in range(N_CORES)]
    return np.concatenate(outs, axis=0).astype(np.float32), res


def kernel(**inputs):
    return run(inputs)[0]



# revision 5
# speedup vs baseline: 1.1044x; 1.1044x over previous
"""Trainium2 Bass kernel for 3-layer GraphSAGE encoder (nn_Encoder_38757784879702).

Strategy (8 NeuronCores, node-partitioned / graph parallel):
  - Nodes sharded 12500/core (padded to 12544 = 98 tiles of 128).
  - Edges assigned to the core owning dst, sorted by (dst tile, src zrow).
    Chunk (t, k) = k-th run of 128 zrow-sorted edges of dst tile t, so a
    chunk's src rows lie in one zrow quantile band.
  - Band-major gather: tiles processed in groups of G=4; for fixed k the
    G tiles' chunks share a band, so one gpsimd dma_gather call (int16
    idx, 32768-row window) serves up to NCHMAX chunks — ~2.5x fewer
    calls than tile-major order (each call has ~1-2.4us fixed cost).
  - Mean aggregation = one-hot matmul on TensorE per chunk; the one-hot
    is generated ON DEVICE (iota vs dloc is_equal on VectorE) instead of
    streaming ~29MB/layer of host tables; 1/deg is applied after the
    psum sum via a per-partition scale.
  - Transform-first: z_l = y_{l-1} @ Wl_l.T computed per-shard then
    AllGather'd (Shared addr space, 4 node blocks to overlap with
    compute) so every core can gather any z row for its edges.
  - Layer-1 gather table (bf16 x in zrow layout) is replicated to every
    core as an input, eliminating the x AllGather entirely.
  - PReLU runs on the Scalar (ACT) engine as a native activation.
  - Inter-layer DRAM intermediates (rt2, w2, rt3) stored bf16.
"""

import sys

sys.path.insert(0, "/opt/trn_rl_repo")

import numpy as np

import concourse.bass as bass
import concourse.bacc as bacc
import concourse.mybir as mybir
import concourse.tile as tile
from concourse.bass_utils import run_bass_kernel_spmd
from concourse.masks import make_identity

F32 = mybir.dt.float32
BF16 = mybir.dt.bfloat16
F32R = mybir.dt.float32r
I16 = mybir.dt.int16

# ---------------------------------------------------------------------------
# Problem geometry (hardcoded: harness contract)
N_NODES = 100000
N_EDGES = 800000
D_IN = 128
D_H = 512
N_CORES = 8

# MM: dense-matmul operand dtype ("bf16"|"f32r").  PRELU: "act" uses the
# Scalar-engine Prelu activation, "max" the max(x, a*x) DVE fallback.
# NCHMAX <= 8: the gather ucode rejects num_idxs > 1024 (verified on HW).
CONFIG = {"MM": "bf16", "PRELU": "act", "G": 4, "NCHMAX": 8}


class Plan:
    """All host-derived geometry + per-core arrays."""

    def __init__(self, n_nodes, n_cores, d_in, d_h, cfg):
        self.cfg = cfg
        self.N = n_nodes
        self.C = n_cores
        self.D_IN = d_in
        self.D_H = d_h
        assert n_nodes % n_cores == 0
        self.NSH = n_nodes // n_cores              # real nodes per core
        self.NT = -(-self.NSH // 128)              # dst tiles per core
        self.SH = self.NT * 128                    # padded nodes per core
        self.NR = self.C * self.SH                 # padded global rows
        self.G = cfg["G"]                          # tiles per agg group
        self.NCHMAX = cfg["NCHMAX"]                # chunks per gather call
        self.K_C = None                            # from data
        # node blocks for chunked AllGather, aligned to G
        assert self.NT == 98
        self.BT = [24, 24, 24, 26]                                # tiles/block
        self.TS = [sum(self.BT[:i]) for i in range(4)]            # tile start
        self.BS = [bt * 128 for bt in self.BT]                    # rows/core/blk
        self.GB = [self.C * sum(self.BS[:i]) for i in range(4)]   # global base
        self.NBK = 4


def _zrow(plan, n):
    r = n // plan.NSH
    loc = n % plan.NSH
    t = loc // 128
    b = np.searchsorted(np.cumsum(plan.BT), t, side="right")
    b = np.minimum(b, plan.NBK - 1)
    gb = np.asarray(plan.GB)[b]
    bs = np.asarray(plan.BS)[b]
    ts = np.asarray(plan.TS)[b]
    return gb + r * bs + (loc - ts * 128)


def preprocess(plan, x, edge_index, weights):
    """Build per-core input maps (numpy only)."""
    import ml_dtypes
    tdnp = ml_dtypes.bfloat16
    mmnp = np.float32 if plan.cfg["MM"] in ("f32", "f32r") else ml_dtypes.bfloat16

    N, C, NSH, SH, NT = plan.N, plan.C, plan.NSH, plan.SH, plan.NT
    G, NCHMAX = plan.G, plan.NCHMAX
    src = np.asarray(edge_index[0], dtype=np.int64)
    dst = np.asarray(edge_index[1], dtype=np.int64)
    x = np.asarray(x, dtype=np.float32)

    deg = np.bincount(dst, minlength=N)
    invdeg = (1.0 / np.maximum(deg, 1)).astype(np.float32)

    # sort edges by (owning core, dst tile, src zrow): chunk (t, k) covers
    # one zrow quantile band, aligned across tiles and cores
    zr_all = _zrow(plan, src)
    core_all = dst // NSH
    tile_all = (dst % NSH) // 128
    order = np.lexsort((zr_all, tile_all, core_all))
    s_dst = dst[order]
    s_zr = zr_all[order]
    core_of = core_all[order]
    tile_of = tile_all[order]

    gkey = core_of * NT + tile_of
    cnt = np.bincount(gkey, minlength=C * NT).reshape(C, NT)
    plan.K_C = K_C = int(-(-cnt.max() // 128))
    # uniform (across cores) chunk count per tile; >=1 so psum gets zeroed
    plan.KT = KT = np.maximum(1, -(-cnt.max(axis=0) // 128)).astype(np.int64)
    WR = plan.WR = min(32768, plan.NR)
    NRR = plan.NR

    starts = np.cumsum(cnt.reshape(-1)) - cnt.reshape(-1)
    rank = np.arange(len(s_dst)) - starts[gkey]
    p = rank % 128
    k = rank // 128
    dloc_e = (s_dst % NSH) - tile_of * 128        # 0..127 within tile

    # cross-core [lo, hi] zrow union per (tile, chunk)
    lo = np.full((C, NT, K_C), 1 << 60, np.int64)
    hi = np.full((C, NT, K_C), -1, np.int64)
    np.minimum.at(lo, (core_of, tile_of, k), s_zr)
    np.maximum.at(hi, (core_of, tile_of, k), s_zr)
    ulo = lo.min(axis=0)
    uhi = hi.max(axis=0)

    # band-major chunk sequence: per group of G tiles, k outer, tile inner
    seq = []
    for g0 in range(0, NT, G):
        tiles = list(range(g0, min(g0 + G, NT)))
        for kk in range(K_C):
            for t in tiles:
                if kk < KT[t]:
                    seq.append((t, kk))
    plan.seq = seq
    seq_pos = {tk: s for s, tk in enumerate(seq)}
    plan.NSEQ = NSEQ = len(seq)

    # greedy windows over the sequence: merge consecutive chunks while the
    # cross-core zrow union fits one WR-row window, cap NCHMAX per call
    flo = np.array([ulo[t, kk] for (t, kk) in seq])
    fhi = np.array([uhi[t, kk] for (t, kk) in seq])
    plan.windows = []                             # (c0_seq, nch, base)
    plan.chunk2win = {}                           # seq pos -> (wi, off)
    kk = 0
    while kk < NSEQ:
        clo, chi = flo[kk], fhi[kk]
        n = 1
        while kk + n < NSEQ and n < NCHMAX:
            nlo = min(clo, flo[kk + n])
            nhi = max(chi, fhi[kk + n])
            b = min(nlo, NRR - WR) if nhi >= 0 else 0
            if nhi - b <= WR - 1 or nhi < 0:
                clo, chi, n = nlo, nhi, n + 1
            else:
                break
        if chi < 0:
            b = 0
        else:
            b = max(0, min(clo, NRR - WR))
            assert chi - b <= WR - 1, "single chunk exceeds int16 window"
        wi = len(plan.windows)
        plan.windows.append((int(kk), int(n), int(b)))
        for c in range(kk, kk + n):
            plan.chunk2win[c] = (wi, c - kk)
        kk += n

    # per-edge window base -> relative idx
    base_of_seq = np.zeros(NSEQ, np.int64)
    for (c0, n, b) in plan.windows:
        base_of_seq[c0:c0 + n] = b
    # vectorized seq position per edge
    spos_lut = np.full((NT, K_C), -1, np.int64)
    for s, (t, kk) in enumerate(seq):
        spos_lut[t, kk] = s
    spos_e = spos_lut[tile_of, k]
    assert spos_e.min() >= 0
    rel = s_zr - base_of_seq[spos_e]
    assert rel.min() >= 0 and rel.max() < WR

    # idx stream: chunk at seq position s owns 8 int16 columns at s*8;
    # edge j of the chunk -> partition j%16, column s*8 + j//16
    # (replicated across the 8 groups of 16 partitions).
    idx_all = np.zeros((C, 16, NSEQ * 8), np.int16)
    icol = spos_e * 8 + p // 16
    idx_all[core_of, p % 16, icol] = rel.astype(np.int16)
    idx_all = np.tile(idx_all, (1, 8, 1))

    # dst-within-tile index per chunk, -1 padding (never matches iota)
    dloc_all = np.full((C, 128, NT * K_C), -1.0, np.float32)
    dloc_all[core_of, p, tile_of * K_C + k] = dloc_e.astype(np.float32)

    # 1/deg per dst row, [128, NT] (partition = row-in-tile)
    invd_all = np.ones((C, 128, NT), np.float32)
    for c in range(C):
        v = np.ones(SH, np.float32)
        v[:NSH] = invdeg[c * NSH:(c + 1) * NSH]
        invd_all[c] = v.reshape(NT, 128).T

    # x shard (padded, f32) for root/residual terms
    xsh = np.zeros((C, SH, plan.D_IN), np.float32)
    xsh[:, :NSH, :] = x.reshape(C, NSH, plan.D_IN)

    # replicated layer-1 gather table in zrow layout (bf16)
    xg = np.zeros((plan.NR, plan.D_IN), tdnp)
    xg[_zrow(plan, np.arange(N))] = x.astype(tdnp)

    def wt_blocks(w, npdt):
        # W [O, I] -> blocks [128, (I/128)*O], block k = W.T[k*128:(k+1)*128, :]
        wt = np.ascontiguousarray(w.T.astype(np.float32))  # [I, O]
        i, o = wt.shape
        return np.ascontiguousarray(
            wt.reshape(i // 128, 128, o).transpose(1, 0, 2).reshape(128, (i // 128) * o)
        ).astype(npdt)

    a_val = float(np.asarray(weights["a"]))
    iota = np.tile(np.arange(128, dtype=np.float32), (128, 1))

    common = {
        "idx": None,
        "wl1t": wt_blocks(weights["Wl1"], mmnp),
        "wr1t": wt_blocks(weights["Wr1"], mmnp),
        "wwt": wt_blocks(weights["Ww"], mmnp),
        "ww2t": wt_blocks(weights["Ww2"], mmnp),
        "wl2t": wt_blocks(weights["Wl2"], mmnp),
        "wr2t": wt_blocks(weights["Wr2"], mmnp),
        "wl3t": wt_blocks(weights["Wl3"], mmnp),
        "wr3t": wt_blocks(weights["Wr3"], mmnp),
        "bl1": np.asarray(weights["bl1"], np.float32).reshape(1, -1).astype(mmnp),
        "bw": np.asarray(weights["bw"], np.float32).reshape(1, -1).astype(mmnp),
        "bw2": np.asarray(weights["bw2"], np.float32).reshape(1, -1).astype(mmnp),
        "bl2": np.asarray(weights["bl2"], np.float32).reshape(1, -1).astype(mmnp),
        "bl3": np.asarray(weights["bl3"], np.float32).reshape(1, -1).astype(mmnp),
        "a_bc": np.full((128, 1), a_val, np.float32),
        "ones_in": np.ones((1, 128), np.float32).astype(mmnp),
        "iota_in": iota,
        "xg": np.ascontiguousarray(xg),
    }
    in_maps = []
    for c in range(C):
        m = dict(common)
        m["idx"] = np.ascontiguousarray(idx_all[c])
        m["dloc"] = np.ascontiguousarray(dloc_all[c])
        m["invd"] = np.ascontiguousarray(invd_all[c])
        m["x_sh"] = np.ascontiguousarray(xsh[c])
        in_maps.append(m)
    return in_maps


def build_program(plan):
    """Emit the SPMD Bass/Tile program (identical for every core)."""
    cfg = plan.cfg
    MM = cfg["MM"]
    MMD = {"f32r": F32R, "bf16": BF16}[MM]
    TD = BF16
    NT, SH, NR, K_C = plan.NT, plan.SH, plan.NR, plan.K_C
    KT = plan.KT
    WR, windows = plan.WR, plan.windows
    chunk2win, NCHMAX, G = plan.chunk2win, plan.NCHMAX, plan.G
    seq = plan.seq
    seq_pos = {tk: s for s, tk in enumerate(seq)}
    D_I, D_Hh = plan.D_IN, plan.D_H
    KB = D_Hh // 128
    RG = [list(range(plan.C))]
    use_act_prelu = cfg["PRELU"] == "act"
    AF = mybir.ActivationFunctionType

    NBK, BT, TS, BS, GB = plan.NBK, plan.BT, plan.TS, plan.BS, plan.GB
    nc = bacc.Bacc("TRN2", target_bir_lowering=False, debug=False,
                   enable_asserts=False, num_devices=plan.C,
                   num_swdge_queues=4)
    qctr = [0]

    def next_q():
        qctr[0] += 1
        return qctr[0] % 4

    # --- I/O ----------------------------------------------------------------
    x_sh = nc.declare_dram_parameter("x_sh", [SH, D_I], F32, isOutput=False)
    xg = nc.declare_dram_parameter("xg", [NR, D_I], TD, isOutput=False)
    idx = nc.declare_dram_parameter("idx", [128, plan.NSEQ * 8], I16,
                                    isOutput=False)
    dloc = nc.declare_dram_parameter("dloc", [128, NT * K_C], F32,
                                     isOutput=False)
    invd = nc.declare_dram_parameter("invd", [128, NT], F32, isOutput=False)
    iota_in = nc.declare_dram_parameter("iota_in", [128, 128], F32,
                                        isOutput=False)
    wnames = ["wl1t", "wr1t", "wwt", "ww2t"]
    wins = {n: nc.declare_dram_parameter(n, [128, D_Hh], MMD, isOutput=False)
            for n in wnames}
    for n in ["wl2t", "wr2t", "wl3t", "wr3t"]:
        wins[n] = nc.declare_dram_parameter(n, [128, KB * D_Hh], MMD,
                                            isOutput=False)
    bnames = ["bl1", "bw", "bw2", "bl2", "bl3"]
    bins = {n: nc.declare_dram_parameter(n, [1, D_Hh], MMD, isOutput=False)
            for n in bnames}
    a_bc = nc.declare_dram_parameter("a_bc", [128, 1], F32, isOutput=False)
    ones_in = nc.declare_dram_parameter("ones_in", [1, 128], MMD,
                                        isOutput=False)
    h3_out = nc.declare_dram_parameter("h3", [SH, D_Hh], F32, isOutput=True)

    with tile.TileContext(nc) as tc:
        with (
            tc.tile_pool(name="dram", bufs=1, space="DRAM") as dpool,
            tc.tile_pool(name="const", bufs=1) as cpool,
            tc.tile_pool(name="gin", bufs=6) as gpool,
            tc.tile_pool(name="ohp", bufs=6) as ohpool,
            tc.tile_pool(name="work", bufs=2) as wk,
            tc.tile_pool(name="psA", bufs=4, space="PSUM") as psA,
            tc.tile_pool(name="psB", bufs=2, space="PSUM") as psB,
            tc.tile_pool(name="psT", bufs=2, space="PSUM") as psT,
        ):
            # --- internal DRAM ---------------------------------------------
            z_loc = {}
            z_full = {}
            for l in (2, 3):
                # Local (not Shared): chunked AllGathers need multiple
                # writers per tensor, which Shared's single-writer
                # scheduling model rejects; chunk overlap wins vs the
                # direct pair-HBM write path.
                z_full[l] = dpool.tile([NR, D_Hh], TD, name=f"z{l}full")
                for b in range(NBK):
                    z_loc[(l, b)] = dpool.tile([BS[b], D_Hh], TD,
                                               name=f"z{l}loc{b}")
            w2_d = dpool.tile([SH, D_Hh], TD, name="w2_d")
            rt2_d = dpool.tile([SH, D_Hh], TD, name="rt2_d")
            rt3_d = dpool.tile([SH, D_Hh], TD, name="rt3_d")

            # --- persistent SBUF -------------------------------------------
            ident = cpool.tile([128, 128], F32, name="ident")
            make_identity(nc, ident[:])
            ones1 = cpool.tile([1, 128], MMD, name="ones1")
            nc.sync.dma_start(out=ones1[:], in_=ones_in[:])
            a_sb = cpool.tile([128, 1], F32, name="a_sb")
            nc.sync.dma_start(out=a_sb[:], in_=a_bc[:])
            iota_sb = cpool.tile([128, 128], F32, name="iota_sb")
            nc.sync.dma_start(out=iota_sb[:], in_=iota_in[:])
            dloc_sb = cpool.tile([128, NT * K_C], F32, name="dloc_sb")
            nc.sync.dma_start(out=dloc_sb[:], in_=dloc[:])
            invd_sb = cpool.tile([128, NT], F32, name="invd_sb")
            nc.sync.dma_start(out=invd_sb[:], in_=invd[:])
            idx_sb = cpool.tile([128, plan.NSEQ * 8], I16, name="idx_sb")
            nc.sync.dma_start(out=idx_sb[:], in_=idx[:])
            wsb = {}
            for n, hh in wins.items():
                kb = 1 if n in wnames else KB
                wsb[n] = cpool.tile([128, kb * D_Hh], MMD, name=f"{n}_sb")
                nc.sync.dma_start(out=wsb[n][:], in_=hh[:])
            bsb = {}
            for n, hh in bins.items():
                bsb[n] = cpool.tile([1, D_Hh], MMD, name=f"{n}_sb")
                nc.sync.dma_start(out=bsb[n][:], in_=hh[:])

            # --- helpers ----------------------------------------------------
            g_tiles = {}

            def emit_window(wi, table, width):
                (c0, nch, b) = windows[wi]
                g = gpool.tile([128, NCHMAX, width], TD, name="g", tag="g")
                ni = nch * 128
                nc.gpsimd.dma_gather(
                    out_ap=g[:, 0:nch, :],
                    in_ap=table[b:b + WR, :],
                    idxs_ap=idx_sb[:, c0 * 8:c0 * 8 + ni // 16],
                    num_idxs=ni,
                    num_idxs_reg=ni,
                    elem_size=width,
                    queue_num=next_q(),
                )
                g_tiles[wi] = g

            def onehot_chunk(t, kk):
                col = t * K_C + kk
                o = ohpool.tile([128, 128], TD, name="oht", tag="oht")
                nc.vector.tensor_scalar(
                    out=o[:], in0=iota_sb[:],
                    scalar1=dloc_sb[:, col:col + 1], scalar2=None,
                    op0=mybir.AluOpType.is_equal)
                return o

            def agg_group(tiles, table, width):
                """Gather + one-hot matmul aggregation for G tiles."""
                ps = [psA.tile([128, width], F32, name="psagg", tag="psagg")
                      for _ in tiles]
                for kk in range(K_C):
                    for ti, t in enumerate(tiles):
                        if kk >= KT[t]:
                            continue
                        s = seq_pos[(t, kk)]
                        wi, off = chunk2win[s]
                        if wi not in g_tiles:
                            emit_window(wi, table, width)
                        o = onehot_chunk(t, kk)
                        nc.tensor.matmul(
                            ps[ti][:, :], lhsT=o[:, :],
                            rhs=g_tiles[wi][:, off, :],
                            start=(kk == 0), stop=(kk == KT[t] - 1))
                return ps

            def transpose128(in_ap, name):
                """[128,128] SBUF f32 -> transposed SBUF tile of MMD."""
                pt = psT.tile([128, 128], F32, name="pt", tag="pt")
                nc.tensor.transpose(out=pt[:], in_=in_ap, identity=ident[:])
                ob = wk.tile([128, 128], MMD, name=name, tag=name)
                nc.scalar.copy(out=ob[:], in_=pt[:])
                return ob

            def prelu(ps_or_sb, name, width=None):
                w = width or ps_or_sb.shape[-1]
                h = wk.tile([128, w], F32, name=name, tag=name)
                if use_act_prelu:
                    nc.scalar.activation(out=h[:], in_=ps_or_sb[:, :],
                                         func=AF.Prelu, alpha=a_sb[:, 0:1])
                else:
                    t2 = wk.tile([128, w], F32, name="t2", tag="t2")
                    nc.vector.tensor_scalar_mul(t2[:], ps_or_sb[:, :],
                                                a_sb[:, 0:1])
                    nc.vector.tensor_tensor(out=h[:], in0=ps_or_sb[:, :],
                                            in1=t2[:],
                                            op=mybir.AluOpType.max)
                return h

            def dense_psum(lhsT_blocks, w_name, bias, name, keep_open=False):
                """sum_k lhsT_k.T @ W_k [+ ones.T @ bias] -> psum [128, D_H]"""
                nblk = len(lhsT_blocks)
                ps = psB.tile([128, D_Hh], F32, name=name, tag="psb")
                nb = nblk + (1 if bias is not None else 0)
                i = 0
                for kk in range(nblk):
                    nc.tensor.matmul(
                        ps[:, :], lhsT=lhsT_blocks[kk][:, :],
                        rhs=wsb[w_name][:, kk * D_Hh:(kk + 1) * D_Hh],
                        start=(i == 0), stop=(i == nb - 1 and not keep_open))
                    i += 1
                if bias is not None:
                    nc.tensor.matmul(
                        ps[:, :], lhsT=ones1[:, :], rhs=bsb[bias][:, :],
                        start=(i == 0), stop=not keep_open)
                return ps

            def zrt_tail(y_sb, t, wl_name, wr_name, bl_name, zl, rt_d):
                """From full-width y tile: z_{l+1} -> z_loc, rt -> DRAM bf16."""
                yT = [transpose128(y_sb[:, kk * 128:(kk + 1) * 128], f"yT{kk}")
                      for kk in range(KB)]
                psz = dense_psum(yT, wl_name, None, "psz")
                z_sb = wk.tile([128, D_Hh], TD, name="z_sb", tag="z_sb")
                nc.scalar.copy(out=z_sb[:], in_=psz[:, :])
                bb = 0
                while bb < NBK - 1 and t >= TS[bb + 1]:
                    bb += 1
                nc.sync.dma_start(
                    out=z_loc[(zl, bb)][(t - TS[bb]) * 128:(t - TS[bb] + 1) * 128, :],
                    in_=z_sb[:, :])
                psr = dense_psum(yT, wr_name, bl_name, "psr")
                rt_sb = wk.tile([128, D_Hh], TD, name="rt_sb", tag="rt_sb")
                nc.scalar.copy(out=rt_sb[:], in_=psr[:, :])
                nc.sync.dma_start(out=rt_d[t * 128:(t + 1) * 128, :], in_=rt_sb[:])

            def agg_scale_add(psa, t, rt_t, name):
                """(psa * invdeg[t]) + rt -> f32 SBUF tile."""
                pre = wk.tile([128, D_Hh], F32, name=name, tag="pre")
                nc.vector.scalar_tensor_tensor(
                    out=pre[:], in0=psa[:, :], scalar=invd_sb[:, t:t + 1],
                    in1=rt_t[:], op0=mybir.AluOpType.mult,
                    op1=mybir.AluOpType.add)
                return pre

            # =================== Layer drains ==============================
            def l1_drain(t, psa):
                x_t = wk.tile([128, D_I], F32, name="x_t", tag="x_t")
                nc.sync.dma_start(out=x_t[:], in_=x_sh[t * 128:(t + 1) * 128, :])
                xT = transpose128(x_t[:, :], "xT")
                agg1 = wk.tile([128, D_I], F32, name="agg1", tag="agg1")
                nc.scalar.activation(out=agg1[:], in_=psa[:, :], func=AF.Copy,
                                     scale=invd_sb[:, t:t + 1])
                aggT = transpose128(agg1[:, :], "aggT")
                psh = dense_psum([aggT], "wl1t", None, "psh", keep_open=True)
                nc.tensor.matmul(psh[:, :], lhsT=xT[:, :],
                                 rhs=wsb["wr1t"][:, :], start=False, stop=False)
                nc.tensor.matmul(psh[:, :], lhsT=ones1[:, :],
                                 rhs=bsb["bl1"][:, :], start=False, stop=True)
                h1 = prelu(psh, "h1", D_Hh)
                psy = dense_psum([xT], "wwt", "bw", "psy")
                y1 = wk.tile([128, D_Hh], F32, name="y1", tag="y1")
                nc.vector.tensor_tensor(out=y1[:], in0=psy[:, :], in1=h1[:],
                                        op=mybir.AluOpType.add)
                psw = dense_psum([xT], "ww2t", "bw2", "psw")
                w2_sb = wk.tile([128, D_Hh], TD, name="w2_sb", tag="w2_sb")
                nc.vector.tensor_tensor(out=w2_sb[:], in0=psw[:, :], in1=h1[:],
                                        op=mybir.AluOpType.add)
                nc.sync.dma_start(out=w2_d[t * 128:(t + 1) * 128, :], in_=w2_sb[:])
                zrt_tail(y1, t, "wl2t", "wr2t", "bl2", 2, rt2_d)

            def l2_drain(t, psa):
                rt_t = wk.tile([128, D_Hh], TD, name="rt_t", tag="rt_t")
                nc.sync.dma_start(out=rt_t[:],
                                  in_=rt2_d[t * 128:(t + 1) * 128, :])
                pre = agg_scale_add(psa, t, rt_t, "pre2")
                h2 = prelu(pre, "h2", D_Hh)
                w2_t = wk.tile([128, D_Hh], TD, name="w2_t", tag="w2_t")
                nc.sync.dma_start(out=w2_t[:],
                                  in_=w2_d[t * 128:(t + 1) * 128, :])
                y2 = wk.tile([128, D_Hh], F32, name="y2", tag="y2")
                nc.vector.tensor_tensor(out=y2[:], in0=h2[:], in1=w2_t[:],
                                        op=mybir.AluOpType.add)
                zrt_tail(y2, t, "wl3t", "wr3t", "bl3", 3, rt3_d)

            def l3_drain(t, psa):
                rt_t = wk.tile([128, D_Hh], TD, name="rt3_t", tag="rt_t")
                nc.sync.dma_start(out=rt_t[:],
                                  in_=rt3_d[t * 128:(t + 1) * 128, :])
                pre = agg_scale_add(psa, t, rt_t, "pre3")
                h3 = prelu(pre, "h3t", D_Hh)
                nc.sync.dma_start(out=h3_out[t * 128:(t + 1) * 128, :],
                                  in_=h3[:])

            def ag_block(l, b):
                nc.gpsimd.collective_compute(
                    "AllGather", mybir.AluOpType.bypass, replica_groups=RG,
                    ins=[z_loc[(l, b)][:].opt()],
                    outs=[z_full[l][GB[b]:GB[b] + plan.C * BS[b], :].opt()])

            # =================== Sweeps ====================================
            def sweep(table, width, drain):
                g_tiles.clear()
                for g0 in range(0, NT, G):
                    tiles = list(range(g0, min(g0 + G, NT)))
                    ps = agg_group(tiles, table, width)
                    for ti, t in enumerate(tiles):
                        drain(t, ps[ti])
                        yield t

            for t in sweep(xg, D_I, l1_drain):
                for b in range(NBK):
                    if t == TS[b] + BT[b] - 1:
                        ag_block(2, b)
            for t in sweep(z_full[2], D_Hh, l2_drain):
                for b in range(NBK):
                    if t == TS[b] + BT[b] - 1:
                        ag_block(3, b)
            for t in sweep(z_full[3], D_Hh, l3_drain):
                pass

    nc.compile()
    return nc


_CACHE = {}


def _get_program(plan):
    key = (plan.N, plan.C, plan.K_C, len(plan.windows),
           tuple(sorted((k, str(v)) for k, v in plan.cfg.items())))
    if key not in _CACHE:
        _CACHE[key] = build_program(plan)
    return _CACHE[key]


def run(inputs, trace=False, **rkw):
    inputs = {k: np.asarray(v) for k, v in inputs.items()}
    x = inputs["x"]
    edge_index = inputs["edge_index"]
    plan = Plan(N_NODES, N_CORES, D_IN, D_H, CONFIG)
    in_maps = preprocess(plan, x, edge_index, inputs)
    nc = _get_program(plan)
    res = run_bass_kernel_spmd(nc, in_maps, core_ids=list(range(N_CORES)),
                               trace=trace, **rkw)
    outs = [res.results[c]["h3"][:plan.NSH] for c in range(N_CORES)]
    return np.concatenate(outs, axis=0).astype(np.float32), res


def kernel(**inputs):
    return run(inputs)[0]


# revision 17
# speedup vs baseline: 1.4610x; 1.3230x over previous
"""Trainium2 Bass kernel for 3-layer GraphSAGE encoder (nn_Encoder_38757784879702).

Strategy (8 NeuronCores, node-partitioned / graph parallel):
  - Nodes sharded 12500/core (padded to 12544 = 98 tiles of 128).
  - Edges assigned to the core owning dst, sorted by (dst tile, src zrow).
    Chunk (t, k) = k-th run of 128 zrow-sorted edges of dst tile t, so a
    chunk's src rows lie in one zrow quantile band.
  - Band-major gather: tiles processed in groups of G=4; for fixed k the
    G tiles' chunks share a band, so one gpsimd dma_gather call (int16
    idx, 32768-row window) serves up to NCHMAX chunks — ~2.5x fewer
    calls than tile-major order (each call has ~1-2.4us fixed cost).
  - Mean aggregation = one-hot matmul on TensorE per chunk; the one-hot
    is generated ON DEVICE (iota vs dloc is_equal on VectorE) instead of
    streaming ~29MB/layer of host tables; 1/deg is applied after the
    psum sum via a per-partition scale.
  - Transform-first: z_l = y_{l-1} @ Wl_l.T computed per-shard then
    AllGather'd (Shared addr space, 4 node blocks to overlap with
    compute) so every core can gather any z row for its edges.
  - Layer-1 gather table (bf16 x in zrow layout) is replicated to every
    core as an input, eliminating the x AllGather entirely.
  - PReLU runs on the Scalar (ACT) engine as a native activation.
  - Inter-layer DRAM intermediates (rt2, w2, rt3) stored bf16.
"""

import sys

sys.path.insert(0, "/opt/trn_rl_repo")

import numpy as np

import concourse.bass as bass
import concourse.bacc as bacc
import concourse.mybir as mybir
import concourse.tile as tile
from concourse.bass_utils import run_bass_kernel_spmd
from concourse.masks import make_identity

F32 = mybir.dt.float32
BF16 = mybir.dt.bfloat16
F32R = mybir.dt.float32r
I16 = mybir.dt.int16

# ---------------------------------------------------------------------------
# Problem geometry (hardcoded: harness contract)
N_NODES = 100000
N_EDGES = 800000
D_IN = 128
D_H = 512
N_CORES = 8

# MM: dense-matmul operand dtype ("bf16"|"f32r").  PRELU: "act" uses the
# Scalar-engine Prelu activation, "max" the max(x, a*x) DVE fallback.
# NCHMAX <= 8: the gather ucode rejects num_idxs > 1024 (verified on HW).
# FP8 "e4": z2/z3 gather tables + their one-hots in float8_e4m3 (halves
# gather + AllGather traffic; one-hot 1.0/0.0 is exact in fp8).
CONFIG = {"MM": "bf16", "PRELU": "act", "G": 4, "NCHMAX": 8, "FP8": "e4"}


class Plan:
    """All host-derived geometry + per-core arrays."""

    def __init__(self, n_nodes, n_cores, d_in, d_h, cfg):
        self.cfg = cfg
        self.N = n_nodes
        self.C = n_cores
        self.D_IN = d_in
        self.D_H = d_h
        assert n_nodes % n_cores == 0
        self.NSH = n_nodes // n_cores              # real nodes per core
        self.NT = -(-self.NSH // 128)              # dst tiles per core
        self.SH = self.NT * 128                    # padded nodes per core
        self.NR = self.C * self.SH                 # padded global rows
        self.G = cfg["G"]                          # tiles per agg group
        self.NCHMAX = cfg["NCHMAX"]                # chunks per gather call
        self.K_C = None                            # from data
        # node blocks for chunked AllGather, aligned to G
        assert self.NT == 98
        self.BT = [24, 24, 24, 26]                                # tiles/block
        self.TS = [sum(self.BT[:i]) for i in range(4)]            # tile start
        self.BS = [bt * 128 for bt in self.BT]                    # rows/core/blk
        self.GB = [self.C * sum(self.BS[:i]) for i in range(4)]   # global base
        self.NBK = 4


def _zrow(plan, n):
    r = n // plan.NSH
    loc = n % plan.NSH
    t = loc // 128
    b = np.searchsorted(np.cumsum(plan.BT), t, side="right")
    b = np.minimum(b, plan.NBK - 1)
    gb = np.asarray(plan.GB)[b]
    bs = np.asarray(plan.BS)[b]
    ts = np.asarray(plan.TS)[b]
    return gb + r * bs + (loc - ts * 128)


def preprocess(plan, x, edge_index, weights):
    """Build per-core input maps (numpy only)."""
    import ml_dtypes
    tdnp = ml_dtypes.bfloat16
    mmnp = np.float32 if plan.cfg["MM"] in ("f32", "f32r") else ml_dtypes.bfloat16

    N, C, NSH, SH, NT = plan.N, plan.C, plan.NSH, plan.SH, plan.NT
    G, NCHMAX = plan.G, plan.NCHMAX
    src = np.asarray(edge_index[0], dtype=np.int64)
    dst = np.asarray(edge_index[1], dtype=np.int64)
    x = np.asarray(x, dtype=np.float32)

    deg = np.bincount(dst, minlength=N)
    invdeg = (1.0 / np.maximum(deg, 1)).astype(np.float32)

    # sort edges by (owning core, dst tile, src zrow): chunk (t, k) covers
    # one zrow quantile band, aligned across tiles and cores
    zr_all = _zrow(plan, src)
    core_all = dst // NSH
    tile_all = (dst % NSH) // 128
    order = np.lexsort((zr_all, tile_all, core_all))
    s_dst = dst[order]
    s_zr = zr_all[order]
    core_of = core_all[order]
    tile_of = tile_all[order]

    gkey = core_of * NT + tile_of
    cnt = np.bincount(gkey, minlength=C * NT).reshape(C, NT)
    plan.K_C = K_C = int(-(-cnt.max() // 128))
    # uniform (across cores) chunk count per tile; >=1 so psum gets zeroed
    plan.KT = KT = np.maximum(1, -(-cnt.max(axis=0) // 128)).astype(np.int64)
    WR = plan.WR = min(32768, plan.NR)
    NRR = plan.NR

    starts = np.cumsum(cnt.reshape(-1)) - cnt.reshape(-1)
    rank = np.arange(len(s_dst)) - starts[gkey]
    p = rank % 128
    k = rank // 128
    dloc_e = (s_dst % NSH) - tile_of * 128        # 0..127 within tile

    # cross-core [lo, hi] zrow union per (tile, chunk)
    lo = np.full((C, NT, K_C), 1 << 60, np.int64)
    hi = np.full((C, NT, K_C), -1, np.int64)
    np.minimum.at(lo, (core_of, tile_of, k), s_zr)
    np.maximum.at(hi, (core_of, tile_of, k), s_zr)
    ulo = lo.min(axis=0)
    uhi = hi.max(axis=0)

    # band-major chunk sequence: per group of G tiles, k outer, tile inner
    seq = []
    for g0 in range(0, NT, G):
        tiles = list(range(g0, min(g0 + G, NT)))
        for kk in range(K_C):
            for t in tiles:
                if kk < KT[t]:
                    seq.append((t, kk))
    plan.seq = seq
    seq_pos = {tk: s for s, tk in enumerate(seq)}
    plan.NSEQ = NSEQ = len(seq)

    # greedy windows over the sequence: merge consecutive chunks while the
    # cross-core zrow union fits one WR-row window, cap NCHMAX per call
    flo = np.array([ulo[t, kk] for (t, kk) in seq])
    fhi = np.array([uhi[t, kk] for (t, kk) in seq])
    plan.windows = []                             # (c0_seq, nch, base)
    plan.chunk2win = {}                           # seq pos -> (wi, off)
    kk = 0
    while kk < NSEQ:
        clo, chi = flo[kk], fhi[kk]
        n = 1
        while kk + n < NSEQ and n < NCHMAX:
            nlo = min(clo, flo[kk + n])
            nhi = max(chi, fhi[kk + n])
            b = min(nlo, NRR - WR) if nhi >= 0 else 0
            if nhi - b <= WR - 1 or nhi < 0:
                clo, chi, n = nlo, nhi, n + 1
            else:
                break
        if chi < 0:
            b = 0
        else:
            b = max(0, min(clo, NRR - WR))
            assert chi - b <= WR - 1, "single chunk exceeds int16 window"
        wi = len(plan.windows)
        plan.windows.append((int(kk), int(n), int(b)))
        for c in range(kk, kk + n):
            plan.chunk2win[c] = (wi, c - kk)
        kk += n

    # per-edge window base -> relative idx
    base_of_seq = np.zeros(NSEQ, np.int64)
    for (c0, n, b) in plan.windows:
        base_of_seq[c0:c0 + n] = b
    # vectorized seq position per edge
    spos_lut = np.full((NT, K_C), -1, np.int64)
    for s, (t, kk) in enumerate(seq):
        spos_lut[t, kk] = s
    spos_e = spos_lut[tile_of, k]
    assert spos_e.min() >= 0
    rel = s_zr - base_of_seq[spos_e]
    assert rel.min() >= 0 and rel.max() < WR

    # idx stream: chunk at seq position s owns 8 int16 columns at s*8;
    # edge j of the chunk -> partition j%16, column s*8 + j//16
    # (replicated across the 8 groups of 16 partitions).
    idx_all = np.zeros((C, 16, NSEQ * 8), np.int16)
    icol = spos_e * 8 + p // 16
    idx_all[core_of, p % 16, icol] = rel.astype(np.int16)
    idx_all = np.tile(idx_all, (1, 8, 1))

    # dst-within-tile index per chunk, -1 padding (never matches iota);
    # bf16 (values -1..127 exact) for 2x DVE throughput on one-hot gen
    dloc_all = np.full((C, 128, NT * K_C), -1.0, tdnp)
    dloc_all[core_of, p, tile_of * K_C + k] = dloc_e.astype(tdnp)

    # 1/deg per dst row, [128, NT] (partition = row-in-tile)
    invd_all = np.ones((C, 128, NT), np.float32)
    for c in range(C):
        v = np.ones(SH, np.float32)
        v[:NSH] = invdeg[c * NSH:(c + 1) * NSH]
        invd_all[c] = v.reshape(NT, 128).T

    # x shard (padded, f32) for root/residual terms
    xsh = np.zeros((C, SH, plan.D_IN), np.float32)
    xsh[:, :NSH, :] = x.reshape(C, NSH, plan.D_IN)

    # replicated layer-1 gather table in zrow layout (bf16)
    xg = np.zeros((plan.NR, plan.D_IN), tdnp)
    xg[_zrow(plan, np.arange(N))] = x.astype(tdnp)

    def wt_blocks(w, npdt):
        # W [O, I] -> blocks [128, (I/128)*O], block k = W.T[k*128:(k+1)*128, :]
        wt = np.ascontiguousarray(w.T.astype(np.float32))  # [I, O]
        i, o = wt.shape
        return np.ascontiguousarray(
            wt.reshape(i // 128, 128, o).transpose(1, 0, 2).reshape(128, (i // 128) * o)
        ).astype(npdt)

    a_val = float(np.asarray(weights["a"]))
    iota = np.tile(np.arange(128), (128, 1)).astype(tdnp)

    common = {
        "idx": None,
        "wl1t": wt_blocks(weights["Wl1"], mmnp),
        "wr1t": wt_blocks(weights["Wr1"], mmnp),
        "wwt": wt_blocks(weights["Ww"], mmnp),
        "ww2t": wt_blocks(weights["Ww2"], mmnp),
        "wl2t": wt_blocks(weights["Wl2"], mmnp),
        "wr2t": wt_blocks(weights["Wr2"], mmnp),
        "wl3t": wt_blocks(weights["Wl3"], mmnp),
        "wr3t": wt_blocks(weights["Wr3"], mmnp),
        "bl1": np.asarray(weights["bl1"], np.float32).reshape(1, -1).astype(mmnp),
        "bw": np.asarray(weights["bw"], np.float32).reshape(1, -1).astype(mmnp),
        "bw2": np.asarray(weights["bw2"], np.float32).reshape(1, -1).astype(mmnp),
        "bl2": np.asarray(weights["bl2"], np.float32).reshape(1, -1).astype(mmnp),
        "bl3": np.asarray(weights["bl3"], np.float32).reshape(1, -1).astype(mmnp),
        "a_bc": np.full((128, 1), a_val, np.float32),
        "ones_in": np.ones((1, 128), np.float32).astype(mmnp),
        "iota_in": iota,
        "xg": np.ascontiguousarray(xg),
    }
    in_maps = []
    for c in range(C):
        m = dict(common)
        m["idx"] = np.ascontiguousarray(idx_all[c])
        m["dloc"] = np.ascontiguousarray(dloc_all[c])
        m["invd"] = np.ascontiguousarray(invd_all[c])
        m["x_sh"] = np.ascontiguousarray(xsh[c])
        in_maps.append(m)
    return in_maps


def build_program(plan):
    """Emit the SPMD Bass/Tile program (identical for every core)."""
    cfg = plan.cfg
    MM = cfg["MM"]
    MMD = {"f32r": F32R, "bf16": BF16}[MM]
    TD = BF16
    FP8D = mybir.dt.float8e4 if cfg.get("FP8") == "e4" else TD
    NT, SH, NR, K_C = plan.NT, plan.SH, plan.NR, plan.K_C
    KT = plan.KT
    WR, windows = plan.WR, plan.windows
    chunk2win, NCHMAX, G = plan.chunk2win, plan.NCHMAX, plan.G
    seq = plan.seq
    seq_pos = {tk: s for s, tk in enumerate(seq)}
    D_I, D_Hh = plan.D_IN, plan.D_H
    KB = D_Hh // 128
    RG = [list(range(plan.C))]
    use_act_prelu = cfg["PRELU"] == "act"
    AF = mybir.ActivationFunctionType

    NBK, BT, TS, BS, GB = plan.NBK, plan.BT, plan.TS, plan.BS, plan.GB
    nc = bacc.Bacc("TRN2", target_bir_lowering=False, debug=False,
                   enable_asserts=False, num_devices=plan.C,
                   num_swdge_queues=4)
    qctr = [0]

    def next_q():
        qctr[0] += 1
        return qctr[0] % 4

    # --- I/O ----------------------------------------------------------------
    x_sh = nc.declare_dram_parameter("x_sh", [SH, D_I], F32, isOutput=False)
    xg = nc.declare_dram_parameter("xg", [NR, D_I], TD, isOutput=False)
    idx = nc.declare_dram_parameter("idx", [128, plan.NSEQ * 8], I16,
                                    isOutput=False)
    dloc = nc.declare_dram_parameter("dloc", [128, NT * K_C], TD,
                                     isOutput=False)
    invd = nc.declare_dram_parameter("invd", [128, NT], F32, isOutput=False)
    iota_in = nc.declare_dram_parameter("iota_in", [128, 128], TD,
                                        isOutput=False)
    wnames = ["wl1t", "wr1t", "wwt", "ww2t"]
    wins = {n: nc.declare_dram_parameter(n, [128, D_Hh], MMD, isOutput=False)
            for n in wnames}
    for n in ["wl2t", "wr2t", "wl3t", "wr3t"]:
        wins[n] = nc.declare_dram_parameter(n, [128, KB * D_Hh], MMD,
                                            isOutput=False)
    bnames = ["bl1", "bw", "bw2", "bl2", "bl3"]
    bins = {n: nc.declare_dram_parameter(n, [1, D_Hh], MMD, isOutput=False)
            for n in bnames}
    a_bc = nc.declare_dram_parameter("a_bc", [128, 1], F32, isOutput=False)
    ones_in = nc.declare_dram_parameter("ones_in", [1, 128], MMD,
                                        isOutput=False)
    h3_out = nc.declare_dram_parameter("h3", [SH, D_Hh], F32, isOutput=True)

    with tile.TileContext(nc) as tc:
        with (
            tc.tile_pool(name="dram", bufs=1, space="DRAM") as dpool,
            tc.tile_pool(name="const", bufs=1) as cpool,
            tc.tile_pool(name="gin", bufs=8) as gpool,
            tc.tile_pool(name="ohp", bufs=8) as ohpool,
            tc.tile_pool(name="work", bufs=2) as wk,
            tc.tile_pool(name="psA", bufs=4, space="PSUM") as psA,
            tc.tile_pool(name="psB", bufs=2, space="PSUM") as psB,
            tc.tile_pool(name="psT", bufs=2, space="PSUM") as psT,
        ):
            # --- internal DRAM ---------------------------------------------
            z_loc = {}
            z_full = {}
            for l in (2, 3):
                # Local (not Shared): chunked AllGathers need multiple
                # writers per tensor, which Shared's single-writer
                # scheduling model rejects; chunk overlap wins vs the
                # direct pair-HBM write path.
                z_full[l] = dpool.tile([NR, D_Hh], FP8D, name=f"z{l}full")
                for b in range(NBK):
                    z_loc[(l, b)] = dpool.tile([BS[b], D_Hh], FP8D,
                                               name=f"z{l}loc{b}")
            w2_d = dpool.tile([SH, D_Hh], TD, name="w2_d")
            rt2_d = dpool.tile([SH, D_Hh], TD, name="rt2_d")
            rt3_d = dpool.tile([SH, D_Hh], TD, name="rt3_d")

            # --- persistent SBUF -------------------------------------------
            ident = cpool.tile([128, 128], F32, name="ident")
            make_identity(nc, ident[:])
            ones1 = cpool.tile([1, 128], MMD, name="ones1")
            nc.sync.dma_start(out=ones1[:], in_=ones_in[:])
            a_sb = cpool.tile([128, 1], F32, name="a_sb")
            nc.sync.dma_start(out=a_sb[:], in_=a_bc[:])
            iota_sb = cpool.tile([128, 128], TD, name="iota_sb")
            nc.sync.dma_start(out=iota_sb[:], in_=iota_in[:])
            dloc_sb = cpool.tile([128, NT * K_C], TD, name="dloc_sb")
            nc.sync.dma_start(out=dloc_sb[:], in_=dloc[:])
            invd_sb = cpool.tile([128, NT], F32, name="invd_sb")
            nc.sync.dma_start(out=invd_sb[:], in_=invd[:])
            idx_sb = cpool.tile([128, plan.NSEQ * 8], I16, name="idx_sb")
            nc.sync.dma_start(out=idx_sb[:], in_=idx[:])
            wsb = {}
            for n, hh in wins.items():
                kb = 1 if n in wnames else KB
                wsb[n] = cpool.tile([128, kb * D_Hh], MMD, name=f"{n}_sb")
                nc.sync.dma_start(out=wsb[n][:], in_=hh[:])
            bsb = {}
            for n, hh in bins.items():
                bsb[n] = cpool.tile([1, D_Hh], MMD, name=f"{n}_sb")
                nc.sync.dma_start(out=bsb[n][:], in_=hh[:])

            # --- helpers ----------------------------------------------------
            g_tiles = {}

            def emit_window(wi, table, width, dt):
                (c0, nch, b) = windows[wi]
                g = gpool.tile([128, NCHMAX, width], dt, name="g", tag="g")
                ni = nch * 128
                nc.gpsimd.dma_gather(
                    out_ap=g[:, 0:nch, :],
                    in_ap=table[b:b + WR, :],
                    idxs_ap=idx_sb[:, c0 * 8:c0 * 8 + ni // 16],
                    num_idxs=ni,
                    num_idxs_reg=ni,
                    elem_size=width,
                    queue_num=next_q(),
                )
                g_tiles[wi] = g

            def onehot_tile(t, dt):
                """All K_C chunk one-hots of a tile in ONE DVE op via
                stride-0 broadcast APs: out[e, k, d] = (dloc[e,k] == d)."""
                o = ohpool.tile([128, K_C, 128], dt, name="oht", tag="oht")
                nc.vector.tensor_tensor(
                    out=o[:, :, :],
                    in0=dloc_sb[:, t * K_C:(t + 1) * K_C]
                        .unsqueeze(2).broadcast_to((128, K_C, 128)),
                    in1=iota_sb[:, :]
                        .unsqueeze(1).broadcast_to((128, K_C, 128)),
                    op=mybir.AluOpType.is_equal)
                return o

            def agg_group(tiles, table, width, dt):
                """Gather + one-hot matmul aggregation for G tiles."""
                ps = [psA.tile([128, width], F32, name="psagg", tag="psagg")
                      for _ in tiles]
                ohs = [onehot_tile(t, dt) for t in tiles]
                for kk in range(K_C):
                    for ti, t in enumerate(tiles):
                        if kk >= KT[t]:
                            continue
                        s = seq_pos[(t, kk)]
                        wi, off = chunk2win[s]
                        if wi not in g_tiles:
                            emit_window(wi, table, width, dt)
                        nc.tensor.matmul(
                            ps[ti][:, :], lhsT=ohs[ti][:, kk, :],
                            rhs=g_tiles[wi][:, off, :],
                            start=(kk == 0), stop=(kk == KT[t] - 1))
                return ps

            def transpose128(in_ap, name):
                """[128,128] SBUF f32 -> transposed SBUF tile of MMD."""
                pt = psT.tile([128, 128], F32, name="pt", tag="pt")
                nc.tensor.transpose(out=pt[:], in_=in_ap, identity=ident[:])
                ob = wk.tile([128, 128], MMD, name=name, tag=name)
                nc.scalar.copy(out=ob[:], in_=pt[:])
                return ob

            def prelu(ps_or_sb, name, width=None):
                w = width or ps_or_sb.shape[-1]
                h = wk.tile([128, w], F32, name=name, tag=name)
                if use_act_prelu:
                    nc.scalar.activation(out=h[:], in_=ps_or_sb[:, :],
                                         func=AF.Prelu, alpha=a_sb[:, 0:1])
                else:
                    t2 = wk.tile([128, w], F32, name="t2", tag="t2")
                    nc.vector.tensor_scalar_mul(t2[:], ps_or_sb[:, :],
                                                a_sb[:, 0:1])
                    nc.vector.tensor_tensor(out=h[:], in0=ps_or_sb[:, :],
                                            in1=t2[:],
                                            op=mybir.AluOpType.max)
                return h

            def dense_psum(lhsT_blocks, w_name, bias, name, keep_open=False):
                """sum_k lhsT_k.T @ W_k [+ ones.T @ bias] -> psum [128, D_H]"""
                nblk = len(lhsT_blocks)
                ps = psB.tile([128, D_Hh], F32, name=name, tag="psb")
                nb = nblk + (1 if bias is not None else 0)
                i = 0
                for kk in range(nblk):
                    nc.tensor.matmul(
                        ps[:, :], lhsT=lhsT_blocks[kk][:, :],
                        rhs=wsb[w_name][:, kk * D_Hh:(kk + 1) * D_Hh],
                        start=(i == 0), stop=(i == nb - 1 and not keep_open))
                    i += 1
                if bias is not None:
                    nc.tensor.matmul(
                        ps[:, :], lhsT=ones1[:, :], rhs=bsb[bias][:, :],
                        start=(i == 0), stop=not keep_open)
                return ps

            def zrt_tail(y_sb, t, wl_name, wr_name, bl_name, zl, rt_d):
                """From full-width y tile: z_{l+1} -> z_loc, rt -> DRAM bf16."""
                yT = [transpose128(y_sb[:, kk * 128:(kk + 1) * 128], f"yT{kk}")
                      for kk in range(KB)]
                psz = dense_psum(yT, wl_name, None, "psz")
                z_sb = wk.tile([128, D_Hh], FP8D, name="z_sb", tag="z_sb")
                nc.scalar.copy(out=z_sb[:], in_=psz[:, :])
                bb = 0
                while bb < NBK - 1 and t >= TS[bb + 1]:
                    bb += 1
                nc.sync.dma_start(
                    out=z_loc[(zl, bb)][(t - TS[bb]) * 128:(t - TS[bb] + 1) * 128, :],
                    in_=z_sb[:, :])
                psr = dense_psum(yT, wr_name, bl_name, "psr")
                rt_sb = wk.tile([128, D_Hh], TD, name="rt_sb", tag="rt_sb")
                nc.scalar.copy(out=rt_sb[:], in_=psr[:, :])
                nc.sync.dma_start(out=rt_d[t * 128:(t + 1) * 128, :], in_=rt_sb[:])

            def agg_scale_add(psa, t, rt_t, name):
                """(psa * invdeg[t]) + rt -> f32 SBUF tile."""
                pre = wk.tile([128, D_Hh], F32, name=name, tag="pre")
                nc.vector.scalar_tensor_tensor(
                    out=pre[:], in0=psa[:, :], scalar=invd_sb[:, t:t + 1],
                    in1=rt_t[:], op0=mybir.AluOpType.mult,
                    op1=mybir.AluOpType.add)
                return pre

            # =================== Layer drains ==============================
            def l1_drain(t, psa):
                x_t = wk.tile([128, D_I], F32, name="x_t", tag="x_t")
                nc.sync.dma_start(out=x_t[:], in_=x_sh[t * 128:(t + 1) * 128, :])
                xT = transpose128(x_t[:, :], "xT")
                agg1 = wk.tile([128, D_I], F32, name="agg1", tag="agg1")
                nc.scalar.activation(out=agg1[:], in_=psa[:, :], func=AF.Copy,
                                     scale=invd_sb[:, t:t + 1])
                aggT = transpose128(agg1[:, :], "aggT")
                psh = dense_psum([aggT], "wl1t", None, "psh", keep_open=True)
                nc.tensor.matmul(psh[:, :], lhsT=xT[:, :],
                                 rhs=wsb["wr1t"][:, :], start=False, stop=False)
                nc.tensor.matmul(psh[:, :], lhsT=ones1[:, :],
                                 rhs=bsb["bl1"][:, :], start=False, stop=True)
                h1 = prelu(psh, "h1", D_Hh)
                psy = dense_psum([xT], "wwt", "bw", "psy")
                y1 = wk.tile([128, D_Hh], F32, name="y1", tag="y1")
                nc.vector.tensor_tensor(out=y1[:], in0=psy[:, :], in1=h1[:],
                                        op=mybir.AluOpType.add)
                psw = dense_psum([xT], "ww2t", "bw2", "psw")
                w2_sb = wk.tile([128, D_Hh], TD, name="w2_sb", tag="w2_sb")
                nc.vector.tensor_tensor(out=w2_sb[:], in0=psw[:, :], in1=h1[:],
                                        op=mybir.AluOpType.add)
                nc.sync.dma_start(out=w2_d[t * 128:(t + 1) * 128, :], in_=w2_sb[:])
                zrt_tail(y1, t, "wl2t", "wr2t", "bl2", 2, rt2_d)

            def l2_drain(t, psa):
                rt_t = wk.tile([128, D_Hh], TD, name="rt_t", tag="rt_t")
                nc.sync.dma_start(out=rt_t[:],
                                  in_=rt2_d[t * 128:(t + 1) * 128, :])
                pre = agg_scale_add(psa, t, rt_t, "pre2")
                h2 = prelu(pre, "h2", D_Hh)
                w2_t = wk.tile([128, D_Hh], TD, name="w2_t", tag="w2_t")
                nc.sync.dma_start(out=w2_t[:],
                                  in_=w2_d[t * 128:(t + 1) * 128, :])
                y2 = wk.tile([128, D_Hh], F32, name="y2", tag="y2")
                nc.vector.tensor_tensor(out=y2[:], in0=h2[:], in1=w2_t[:],
                                        op=mybir.AluOpType.add)
                zrt_tail(y2, t, "wl3t", "wr3t", "bl3", 3, rt3_d)

            def l3_drain(t, psa):
                rt_t = wk.tile([128, D_Hh], TD, name="rt3_t", tag="rt_t")
                nc.sync.dma_start(out=rt_t[:],
                                  in_=rt3_d[t * 128:(t + 1) * 128, :])
                pre = agg_scale_add(psa, t, rt_t, "pre3")
                h3 = prelu(pre, "h3t", D_Hh)
                nc.sync.dma_start(out=h3_out[t * 128:(t + 1) * 128, :],
                                  in_=h3[:])

            def ag_block(l, b):
                nc.gpsimd.collective_compute(
                    "AllGather", mybir.AluOpType.bypass, replica_groups=RG,
                    ins=[z_loc[(l, b)][:].opt()],
                    outs=[z_full[l][GB[b]:GB[b] + plan.C * BS[b], :].opt()])

            # =================== Sweeps ====================================
            def sweep(table, width, dt, drain):
                g_tiles.clear()
                for g0 in range(0, NT, G):
                    tiles = list(range(g0, min(g0 + G, NT)))
                    ps = agg_group(tiles, table, width, dt)
                    for ti, t in enumerate(tiles):
                        drain(t, ps[ti])
                        yield t

            for t in sweep(xg, D_I, TD, l1_drain):
                for b in range(NBK):
                    if t == TS[b] + BT[b] - 1:
                        ag_block(2, b)
            for t in sweep(z_full[2], D_Hh, FP8D, l2_drain):
                for b in range(NBK):
                    if t == TS[b] + BT[b] - 1:
                        ag_block(3, b)
            for t in sweep(z_full[3], D_Hh, FP8D, l3_drain):
                pass

    nc.compile()
    return nc


_CACHE = {}


def _get_program(plan):
    key = (plan.N, plan.C, plan.K_C, len(plan.windows),
           tuple(sorted((k, str(v)) for k, v in plan.cfg.items())))
    if key not in _CACHE:
        _CACHE[key] = build_program(plan)
    return _CACHE[key]


def run(inputs, trace=False, **rkw):
    inputs = {k: np.asarray(v) for k, v in inputs.items()}
    x = inputs["x"]
    edge_index = inputs["edge_index"]
    plan = Plan(N_NODES, N_CORES, D_IN, D_H, CONFIG)
    in_maps = preprocess(plan, x, edge_index, inputs)
    nc = _get_program(plan)
    res = run_bass_kernel_spmd(nc, in_maps, core_ids=list(range(N_CORES)),
                               trace=trace, **rkw)
    outs = [res.results[c]["h3"][:plan.NSH] for c in range(N_CORES)]
    return np.concatenate(outs, axis=0).astype(np.float32), res


def kernel(**inputs):
    return run(inputs)[0]
